# revision 37
# baseline (speedup 1.0000x reference)
"""Causal self-attention with RoPE, tensor-parallel over (batch, head-group)
across 8 NeuronCores.

Sharding: core c = 4*b + g handles batch b (of 2) and head group g (of 4),
i.e. heads 4g..4g+3.  Each core computes q/k projections in transposed
layout [head_dim, seq], v in natural layout [seq, head_dim], applies RoPE,
runs causal attention without max-subtraction (scores are O(3), exp is safe
in fp32), and emits a partial output projection.  The host sums the 4
per-head-group partials per batch.

Optimizations over the simple schedule:
 - All matmul operands fp16 (full PE rate, f32 PSUM accumulation).
 - PSUM eviction (QKV waves, out-proj) on the otherwise-idle ACT engine
   (fp16 cast); RoPE runs on DVE in fp16 2x mode with 0-stride
   head-broadcast APs for the cos/sin tables.
 - Softmax denominator: DVE pairwise et sums (fp16) + one ones-matmul per
   pair instead of two -> halves the PE cost of the denominator.
 - Block-causal with partial diagonal tiles: scores/exp/AV only stream the
   i-range at-or-below the diagonal 128-tile; out-of-range et is memset and
   the 128-wide triangle masked, so PE/ACT cols match the exact tril count.
 - Out-projection interleaved into the attention stream (4 PSUM groups
   after each head-block) so the PE fills attention's ACT/DVE-bound gaps;
   stores are fp16 (host accumulates partials in f32).
 - Head DMAs split so the first matmul starts ~6us earlier; bulk loads
   issued on the scalar-engine HWDGE ring in parallel with sync's.
"""

import sys
from contextlib import ExitStack

sys.path.insert(0, "/opt/trn_rl_repo")

import numpy as np

import concourse.bass as bass
import concourse.tile as tile
from concourse import bacc, bass_isa, mybir

B, S, D, H, HD = 2, 2048, 2048, 16, 128
NCORES = 8
HPC = H // 4  # heads per core = 4
DG = HPC * HD  # 512 cols per head group
P = 128
SB = 512  # s-block (matmul free dim)
NSB = S // SB  # 4
NDT = D // P  # 16 contraction tiles of the model dim
NST = S // P  # 16 seq tiles
F32 = mybir.dt.float32
MMDT = mybir.dt.float16
MMNP = np.float16
SCALE = 1.0 / float(np.sqrt(HD))
EXP = mybir.ActivationFunctionType.Exp


def _bc_heads(ap2d, nh=HPC):
    """[rows, cols] AP -> [rows, nh, cols] with a 0-stride head dim."""
    return ap2d.unsqueeze(1).broadcast_to((ap2d.shape[0], nh, ap2d.shape[1]))


def _build_program(with_qkv_bias: bool):
    nc = bacc.Bacc("TRN2", target_bir_lowering=False, debug=False,
                   num_devices=NCORES)
    # all big inputs pre-tiled partition-major on the host so each DMA is
    # one contiguous run per partition (few descriptors, full spray)
    xH = nc.dram_tensor("xH", [P, NSB, NDT, SB], MMDT,
                        kind="ExternalInput").ap()
    wq = nc.dram_tensor("wq", [P, NDT, DG], MMDT, kind="ExternalInput").ap()
    wk = nc.dram_tensor("wk", [P, NDT, DG], MMDT, kind="ExternalInput").ap()
    wv = nc.dram_tensor("wv", [P, NDT, DG], MMDT, kind="ExternalInput").ap()
    wo = nc.dram_tensor("wo", [P, HPC, D], MMDT, kind="ExternalInput").ap()
    cosT = nc.dram_tensor("cosT", [P, S], MMDT, kind="ExternalInput").ap()
    sinST = nc.dram_tensor("sinST", [P, S], MMDT, kind="ExternalInput").ap()
    mask128 = nc.dram_tensor("mask128", [P, P], MMDT,
                             kind="ExternalInput").ap()
    onesd = nc.dram_tensor("ones", [P, P], MMDT, kind="ExternalInput").ap()
    if with_qkv_bias:
        bqr = nc.dram_tensor("bqrope", [P, HPC, S], F32,
                             kind="ExternalInput").ap()
        bkr = nc.dram_tensor("bkrope", [P, HPC, S], F32,
                             kind="ExternalInput").ap()
        bv128 = nc.dram_tensor("bv128", [P, DG], F32,
                               kind="ExternalInput").ap()
    out = nc.dram_tensor("out", [S, D], MMDT, kind="ExternalOutput").ap()

    with tile.TileContext(nc) as tc:
        with ExitStack() as top:
            # ---- persistent tiles ----
            qkT_pool = top.enter_context(tc.tile_pool(name="qkT", bufs=1))
            qT = qkT_pool.tile([P, HPC, S], MMDT, tag="qT")
            kT = qkT_pool.tile([P, HPC, S], MMDT, tag="kT")
            v_pool = top.enter_context(tc.tile_pool(name="vp", bufs=1))
            vN = v_pool.tile([P, NST, DG], MMDT, tag="vN")
            oT_pool = top.enter_context(tc.tile_pool(name="oTp", bufs=1))
            oT = oT_pool.tile([P, HPC, S], MMDT, tag="oT")
            wopool = top.enter_context(tc.tile_pool(name="wo", bufs=1))
            wo_t = wopool.tile([P, HPC, D], MMDT, tag="wo")
            cpool = top.enter_context(tc.tile_pool(name="cs", bufs=1))
            cos_t = cpool.tile([P, S], MMDT, tag="cos")
            sin_t = cpool.tile([P, S], MMDT, tag="sin")
            mpool = top.enter_context(tc.tile_pool(name="msk", bufs=1))
            mask_t = mpool.tile([P, P], MMDT, tag="mask")
            ones_t = mpool.tile([P, P], MMDT, tag="ones")

            # ---- phase A: q/k/v projections (sb3's v/k waves deferred
            # into attention as PE filler for its ACT-bound stretches) ----
            defer_vk = False
            a_sbuf = top if defer_vk else ExitStack()
            wpool = a_sbuf.enter_context(tc.tile_pool(name="wqkv", bufs=1))
            wq_t = wpool.tile([P, NDT, DG], MMDT, tag="wq")
            wk_t = wpool.tile([P, NDT, DG], MMDT, tag="wk")
            wv_t = wpool.tile([P, NDT, DG], MMDT, tag="wv")
            xpool = a_sbuf.enter_context(tc.tile_pool(name="xs", bufs=2))
            pcpool = a_sbuf.enter_context(tc.tile_pool(name="pc", bufs=2))
            tmppool = a_sbuf.enter_context(tc.tile_pool(name="rt", bufs=2))
            if with_qkv_bias:
                bpool = a_sbuf.enter_context(tc.tile_pool(name="bqk", bufs=1))
                bqr_t = bpool.tile([P, HPC, S], F32, tag="bqr")
                bkr_t = bpool.tile([P, HPC, S], F32, tag="bkr")
                bv_t = bpool.tile([P, DG], F32, tag="bv")

            # DMA launches in consumption order; the sync ring streams the
            # weights while the scalar ring streams x and the tables
            xs0 = xpool.tile([P, NDT, SB], MMDT, tag="xs", name="x_0")
            xs1 = xpool.tile([P, NDT, SB], MMDT, tag="xs", name="x_1")
            nc.sync.dma_start(wq_t[:, 0:2, :], wq[:, 0:2, :])
            nc.sync.dma_start(wq_t[:, 2:6, :], wq[:, 2:6, :])
            nc.sync.dma_start(wq_t[:, 6:11, :], wq[:, 6:11, :])
            nc.sync.dma_start(wq_t[:, 11:16, :], wq[:, 11:16, :])
            nc.sync.dma_start(wk_t[:, 0:8, :], wk[:, 0:8, :])
            nc.sync.dma_start(wk_t[:, 8:16, :], wk[:, 8:16, :])
            nc.sync.dma_start(wv_t[:, 0:8, :], wv[:, 0:8, :])
            nc.sync.dma_start(wv_t[:, 8:16, :], wv[:, 8:16, :])
            nc.scalar.dma_start(xs0[:, 0:2, :], xH[:, 0, 0:2, :])
            nc.scalar.dma_start(xs0[:, 2:6, :], xH[:, 0, 2:6, :])
            nc.scalar.dma_start(xs0[:, 6:11, :], xH[:, 0, 6:11, :])
            nc.scalar.dma_start(xs0[:, 11:16, :], xH[:, 0, 11:16, :])
            nc.scalar.dma_start(cos_t[:], cosT[:])
            nc.scalar.dma_start(sin_t[:], sinST[:])
            nc.scalar.dma_start(mask_t[:], mask128[:])
            nc.scalar.dma_start(ones_t[:], onesd[:])
            nc.scalar.dma_start(xs1[:], xH[:, 1, :, :])
            nc.scalar.dma_start(wo_t[:], wo[:])
            if with_qkv_bias:
                nc.scalar.dma_start(bqr_t[:], bqr[:])
                nc.scalar.dma_start(bkr_t[:], bkr[:])
                nc.scalar.dma_start(bv_t[:], bv128[:])

            def rope(pc2d, dst_ap, ssl, bias_ap):
                """dst = pc*cos + rot(pc)*sinS via the pre-rotated sin table
                (both SB operands of each mul share a base partition)."""
                tmp = tmppool.tile([P, HPC, SB], MMDT, tag="rt")
                nc.vector.tensor_mul(tmp[0:64, 0, :], pc2d[64:128, :],
                                     sin_t[64:128, ssl])
                nc.vector.tensor_mul(tmp[64:128, 0, :], pc2d[0:64, :],
                                     sin_t[0:64, ssl])
                nc.vector.tensor_mul(dst_ap, pc2d[:], cos_t[:, ssl])
                nc.vector.tensor_add(dst_ap, dst_ap, tmp[:, 0, :])
                if bias_ap is not None:
                    nc.vector.tensor_add(dst_ap, dst_ap, bias_ap)

            with ExitStack() as actx:
                psA = actx.enter_context(
                    tc.tile_pool(name="psA", bufs=2, space="PSUM"))

                def qk_wave(sb, xs, wname, w_t, dst):
                    ssl = bass.ts(sb, SB)
                    ps = psA.tile([P, HPC, SB], F32, tag="psA",
                                  name=f"ps{wname}_{sb}")
                    for dt in range(NDT):
                        for h in range(HPC):
                            nc.tensor.matmul(
                                ps[:, h, :], w_t[:, dt, bass.ts(h, P)],
                                xs[:, dt, :],
                                start=(dt == 0), stop=(dt == NDT - 1))
                    pc = pcpool.tile([P, HPC, SB], MMDT, tag="pc")
                    nc.scalar.copy(pc[:], ps[:])
                    tmp = tmppool.tile([P, HPC, SB], MMDT, tag="rt")
                    nc.vector.tensor_mul(tmp[0:64, :, :], pc[64:128, :, :],
                                         _bc_heads(sin_t[64:128, ssl]))
                    nc.vector.tensor_mul(tmp[64:128, :, :], pc[0:64, :, :],
                                         _bc_heads(sin_t[0:64, ssl]))
                    dst_ap = dst[:, :, ssl]
                    nc.vector.tensor_mul(dst_ap, pc[:],
                                         _bc_heads(cos_t[:, ssl]))
                    nc.vector.tensor_add(dst_ap, dst_ap, tmp[:])
                    if with_qkv_bias:
                        bt = bqr_t if wname == "q" else bkr_t
                        nc.vector.tensor_add(dst_ap, dst_ap, bt[:, :, ssl])

                def v_wave(sb, xs):
                    psv = psA.tile([P, HPC, DG], F32, tag="psA",
                                   name=f"psv_{sb}")
                    for dt in range(NDT):
                        for j in range(4):
                            nc.tensor.matmul(
                                psv[:, j, :], xs[:, dt, bass.ts(j, P)],
                                wv_t[:, dt, :],
                                start=(dt == 0), stop=(dt == NDT - 1))
                    vdst = vN[:, bass.ts(sb, 4), :]
                    if sb == NSB - 1:
                        # last A-wave eviction on DVE so ACT is free for
                        # attention's first exps right at phase-A end
                        nc.vector.tensor_copy(vdst, psv[:])
                    else:
                        nc.scalar.copy(vdst, psv[:])
                    if with_qkv_bias:
                        nc.vector.tensor_add(
                            vdst, vdst,
                            bv_t[:].unsqueeze(1).broadcast_to((P, 4, DG)))

                n_full = NSB - 1 if defer_vk else NSB
                xs_tiles = {0: xs0, 1: xs1}
                for sb in range(NSB):
                    xs = xs_tiles.pop(sb)
                    xs_tiles[sb] = xs
                    if sb >= 1 and sb + 1 < NSB:
                        xn = xpool.tile([P, NDT, SB], MMDT, tag="xs",
                                        name=f"x_{sb + 1}")
                        nc.sync.dma_start(xn[:], xH[:, sb + 1, :, :])
                        xs_tiles[sb + 1] = xn
                    if sb < n_full:
                        qk_wave(sb, xs, "q", wq_t, qT)
                        qk_wave(sb, xs, "k", wk_t, kT)
                        v_wave(sb, xs)
                xs3 = xs_tiles[NSB - 1]

            if not defer_vk:
                # nothing in attention needs the A-phase SBUF pools
                a_sbuf.close()

            # ---- phase C+D: causal attention, with the deferred sb3 v/k
            # waves and the out-projection spliced in as PE filler ----
            with ExitStack() as cctx:
                etpool = cctx.enter_context(tc.tile_pool(name="et",
                                                          bufs=12))
                espool = cctx.enter_context(tc.tile_pool(name="es", bufs=8))
                rcpool = cctx.enter_context(tc.tile_pool(name="rc", bufs=4))
                obpool = cctx.enter_context(tc.tile_pool(name="ob", bufs=8))
                # psV first: it lands on the banks of A's final (v2) wave
                # whose gpsimd eviction clears shortly after A ends; psS
                # gets k2's banks which are free immediately
                psV = cctx.enter_context(
                    tc.tile_pool(name="psV", bufs=4, space="PSUM"))
                psS = cctx.enter_context(
                    tc.tile_pool(name="psS", bufs=2, space="PSUM"))

                pending = []
                evict_tog = [0]

                def v3_chunk(j):
                    def run():
                        psv = psV.tile([P, DG], F32, tag="psV",
                                       name=f"psv3_{j}")
                        for dt in range(NDT):
                            nc.tensor.matmul(
                                psv[:], xs3[:, dt, bass.ts(j, P)],
                                wv_t[:, dt, :],
                                start=(dt == 0), stop=(dt == NDT - 1))
                        nc.vector.tensor_copy(vN[:, 4 * (NSB - 1) + j, :],
                                              psv[:])
                    return run

                def qk3_chunk(h, w_t, dstT, nm):
                    def run():
                        ssl = bass.ts(NSB - 1, SB)
                        psk = psV.tile([P, SB], F32, tag="psV",
                                       name=f"ps{nm}3_{h}")
                        for dt in range(NDT):
                            nc.tensor.matmul(
                                psk[:], w_t[:, dt, bass.ts(h, P)],
                                xs3[:, dt, :],
                                start=(dt == 0), stop=(dt == NDT - 1))
                        pc = pcpool.tile([P, HPC, SB], MMDT, tag="pc")
                        nc.scalar.copy(pc[:, 0, :], psk[:])
                        rope(pc[:, 0, :], dstT[:, h, ssl], ssl, None)
                    return run

                if defer_vk:
                    for j in range(4):
                        pending.append(v3_chunk(j))
                        pending.append(qk3_chunk(j, wk_t, kT, "k"))
                    for h in range(HPC):
                        pending.append(qk3_chunk(h, wq_t, qT, "q"))

                def d_group(st, eb):
                    def run(act_evict=False):
                        pe = psV.tile([P, SB], F32, tag="psV",
                                      name=f"pe_{st}_{eb}")
                        for hh in range(HPC):
                            nc.tensor.matmul(
                                pe[:], oT[:, hh, bass.ts(st, P)],
                                wo_t[:, hh, bass.ts(eb, SB)],
                                start=(hh == 0), stop=(hh == HPC - 1))
                        # eviction on DVE: the ACT queue must stay clear
                        # for exps (attention's binding engine); the final
                        # drain alternates onto the then-idle ACT
                        ob = obpool.tile([P, SB], MMDT, tag="ob")
                        if act_evict:
                            nc.scalar.copy(ob[:], pe[:])
                        else:
                            nc.vector.tensor_copy(ob[:], pe[:])
                        nc.sync.dma_start(
                            out[bass.ts(st, P), bass.ts(eb, SB)], ob[:])
                    return run

                def emit_d(n, act_alt=False):
                    for i in range(min(n, len(pending))):
                        pending.pop(0)(act_alt and i % 2 == 1)

                for ib in (0, 1, 2, 3):
                    isl = bass.ts(ib, SB)
                    npair = 2 * (ib + 1)
                    for h in range(HPC):
                        po = psV.tile([P, SB], F32, tag="psV",
                                      name=f"po_{h}_{ib}")
                        pd = psV.tile([P, SB], F32, tag="psV",
                                      name=f"pd_{h}_{ib}")

                        def scores_exp(pt, h=h, ib=ib):
                            pss = psS.tile([P, 2, SB], F32, tag="psS",
                                           name=f"pss_{h}_{ib}_{pt}")
                            et = etpool.tile([P, 2, SB], MMDT, tag="et",
                                             name=f"et_{h}_{ib}_{pt}")
                            for t in range(2):
                                jt = 2 * pt + t
                                o = jt - 4 * ib
                                od = o * P if o > 0 else 0
                                nc.tensor.matmul(
                                    pss[:, t, od:],
                                    kT[:, h, bass.ts(jt, P)],
                                    qT[:, h, ib * SB + od:(ib + 1) * SB],
                                    start=True, stop=True)
                            if 2 * pt + 1 - 4 * ib < 0:
                                nc.scalar.activation(et[:], pss[:], EXP,
                                                     scale=SCALE)
                            else:
                                for t in range(2):
                                    od = (2 * pt + t - 4 * ib) * P
                                    nc.scalar.activation(et[:, t, od:],
                                                         pss[:, t, od:],
                                                         EXP, scale=SCALE)
                                    if od:
                                        nc.vector.memset(et[:, t, 0:od], 0.0)
                                    nc.vector.tensor_mul(
                                        et[:, t, od:od + P],
                                        et[:, t, od:od + P], mask_t[:])
                            es = espool.tile([P, SB], MMDT, tag="es")
                            nc.vector.tensor_add(es[:], et[:, 0, :],
                                                 et[:, 1, :])
                            return et, es

                        def consume(pt, et, es, prev_es, first, last,
                                    h=h, ib=ib):
                            for t in range(2):
                                jt = 2 * pt + t
                                o = jt - 4 * ib
                                od = o * P if o > 0 else 0
                                nc.tensor.matmul(
                                    po[:, od:], vN[:, jt, bass.ts(h, P)],
                                    et[:, t, od:],
                                    start=(first and t == 0),
                                    stop=(last and t == 1))
                            # one ones-matmul per TWO pairs: fold the earlier
                            # pair's sum into this one on DVE first
                            if pt % 2 == 1:
                                nc.vector.tensor_add(es[:], es[:], prev_es[:])
                                nc.tensor.matmul(pd[:], ones_t[:], es[:],
                                                 start=(pt == 1), stop=last)

                        data = {0: scores_exp(0)}
                        prev_es = None
                        for pt in range(npair):
                            if pt + 1 < npair:
                                data[pt + 1] = scores_exp(pt + 1)
                            et, es = data.pop(pt)
                            consume(pt, et, es, prev_es,
                                    pt == 0, pt == npair - 1)
                            prev_es = es

                        # pd is already partition-replicated by the
                        # ones-matmul, so the reciprocal runs on all 128
                        # partitions directly (no broadcast needed)
                        recs = rcpool.tile([P, SB], F32, tag="rc")
                        nc.vector.reciprocal_approx_fast(recs[:], pd[:])
                        nc.vector.tensor_mul(oT[:, h, isl], po[:], recs[:])
                        emit_d(4)
                    # drain the leftovers before queueing this ib's out-proj
                    emit_d(len(pending))
                    for st in range(4 * ib, 4 * ib + 4):
                        for eb in range(NSB):
                            pending.append(d_group(st, eb))
                emit_d(len(pending), act_alt=True)

    nc.compile()
    return nc


def _rot_cols(w):
    """rotate_half applied to the last axis (head-dim columns) of w."""
    r = np.empty_like(w)
    r[..., : HD // 2] = -w[..., HD // 2:]
    r[..., HD // 2:] = w[..., : HD // 2]
    return r


def _host_inputs(x, cos, sin, qkv_w, qkv_b, with_qkv_bias):
    """Build the 8 per-core input maps."""
    # signed sin, transposed: sinS[d] = -sin[d] for d<64 else +sin[d]
    sinS = sin.copy()
    sinS[:, : HD // 2] *= -1.0
    cosT = np.ascontiguousarray(cos.T).astype(MMNP)
    # partition-rotated: sinROT[p] = sinS.T[(p + 64) % 128]
    sinST = np.ascontiguousarray(np.roll(sinS.T, 64, axis=0)).astype(MMNP)
    jj = np.arange(P)[:, None]
    ii = np.arange(P)[None, :]
    mask128 = (jj <= ii).astype(MMNP)
    ones = np.ones((P, P), dtype=MMNP)

    def tile_w(w2d):
        # [D, DG] -> [P, NDT, DG] with w[dt*P+p, n] at [p, dt, n]
        return np.ascontiguousarray(
            w2d.reshape(NDT, P, DG).transpose(1, 0, 2)).astype(MMNP)

    # x tiled: xH[p, sb, dt, s'] = x[sb*SB+s', dt*P+p]
    xHb = [np.ascontiguousarray(
        x[b].reshape(NSB, SB, NDT, P).transpose(3, 0, 2, 1)).astype(MMNP)
        for b in range(B)]
    qkv_w16 = qkv_w
    in_maps = []
    for c in range(NCORES):
        b, g = divmod(c, 4)
        cols = slice(g * DG, (g + 1) * DG)
        im = {
            "xH": xHb[b],
            "wq": tile_w(qkv_w16[:, cols]),
            "wk": tile_w(qkv_w16[:, D:][:, cols]),
            "wv": tile_w(qkv_w16[:, 2 * D:][:, cols]),
            "wo": None,  # filled by caller (needs out_w)
            "cosT": cosT,
            "sinST": sinST,
            "mask128": mask128,
            "ones": ones,
        }
        if with_qkv_bias:
            bq = qkv_b[cols]
            bk = qkv_b[D:][cols]
            bv = qkv_b[2 * D:][cols]
            # roped bias, transposed per head: [HD, HPC, S]
            def rope_bias(bvec):
                r = np.empty((P, HPC, S), dtype=np.float32)
                for h in range(HPC):
                    bh = bvec[h * HD:(h + 1) * HD]  # [HD]
                    rb = _rot_cols(bh[None, :])[0]
                    # b*cos + rot(b)*sin, as [HD, S]
                    r[:, h, :] = (bh[None, :] * cos + rb[None, :] * sin).T
                return r
            im["bqrope"] = rope_bias(bq)
            im["bkrope"] = rope_bias(bk)
            im["bv128"] = np.tile(bv[None, :], (P, 1)).astype(np.float32)
        in_maps.append(im)
    return in_maps


_CACHED = {}


def _get_program(with_qkv_bias):
    if with_qkv_bias not in _CACHED:
        _CACHED[with_qkv_bias] = _build_program(with_qkv_bias)
    return _CACHED[with_qkv_bias]


def run_on_cores(in_maps, profile_dir=None):
    """Execute the prebuilt program on 8 cores; optionally capture NTFF."""
    from concourse import bass2jax
    with_qkv_bias = "bqrope" in in_maps[0]
    nc = _get_program(with_qkv_bias)
    if profile_dir is not None:
        from trn_agent_boot.trn_boot import _ntff_profile_via_ctypes
        hook = _ntff_profile_via_ctypes("/opt/axon/libaxon_pjrt.so")
        with hook(profile_dir, [0]):
            results = bass2jax.run_bass_via_pjrt(nc, in_maps, n_cores=NCORES)
    else:
        results = bass2jax.run_bass_via_pjrt(nc, in_maps, n_cores=NCORES)
    return results


def kernel(x, cos, sin, qkv_w, qkv_b, out_w, out_b, _profile_dir=None):
    x = np.asarray(x, dtype=np.float32)
    cos = np.asarray(cos, dtype=np.float32)
    sin = np.asarray(sin, dtype=np.float32)
    qkv_w = np.asarray(qkv_w, dtype=np.float32)
    qkv_b = np.asarray(qkv_b, dtype=np.float32)
    out_w = np.asarray(out_w, dtype=np.float32)
    out_b = np.asarray(out_b, dtype=np.float32)

    with_qkv_bias = bool(np.any(qkv_b != 0))
    in_maps = _host_inputs(x, cos, sin, qkv_w, qkv_b, with_qkv_bias)
    for c in range(NCORES):
        g = c % 4
        # [DG, D] -> [P, HPC, D] with wo[h*P+p, d] at [p, h, d]
        in_maps[c]["wo"] = np.ascontiguousarray(
            out_w[g * DG:(g + 1) * DG, :].reshape(HPC, P, D)
            .transpose(1, 0, 2)).astype(MMNP)

    results = run_on_cores(in_maps, profile_dir=_profile_dir)

    final = np.zeros((B, S, D), dtype=np.float32)
    for c in range(NCORES):
        b = c // 4
        final[b] += results[c]["out"]
    final += out_b[None, None, :]
    return final


# revision 38
# speedup vs baseline: 1.0140x; 1.0140x over previous
"""Causal self-attention with RoPE, tensor-parallel over (batch, head-group)
across 8 NeuronCores.

Sharding: core c = 4*b + g handles batch b (of 2) and head group g (of 4),
i.e. heads 4g..4g+3.  Each core computes q/k projections in transposed
layout [head_dim, seq], v in natural layout [seq, head_dim], applies RoPE,
runs causal attention without max-subtraction (scores are O(3), exp is safe
in fp32), and emits a partial output projection.  The host sums the 4
per-head-group partials per batch.

Optimizations over the simple schedule:
 - All matmul operands fp16 (full PE rate, f32 PSUM accumulation).
 - PSUM eviction (QKV waves, out-proj) on the otherwise-idle ACT engine
   (fp16 cast); RoPE runs on DVE in fp16 2x mode with 0-stride
   head-broadcast APs for the cos/sin tables.
 - Softmax denominator: DVE pairwise et sums (fp16) + one ones-matmul per
   pair instead of two -> halves the PE cost of the denominator.
 - Block-causal with partial diagonal tiles: scores/exp/AV only stream the
   i-range at-or-below the diagonal 128-tile; out-of-range et is memset and
   the 128-wide triangle masked, so PE/ACT cols match the exact tril count.
 - Out-projection interleaved into the attention stream (4 PSUM groups
   after each head-block) so the PE fills attention's ACT/DVE-bound gaps;
   stores are fp16 (host accumulates partials in f32).
 - Head DMAs split so the first matmul starts ~6us earlier; bulk loads
   issued on the scalar-engine HWDGE ring in parallel with sync's.
"""

import sys
from contextlib import ExitStack

sys.path.insert(0, "/opt/trn_rl_repo")

import numpy as np

import concourse.bass as bass
import concourse.tile as tile
from concourse import bacc, bass_isa, mybir

B, S, D, H, HD = 2, 2048, 2048, 16, 128
NCORES = 8
HPC = H // 4  # heads per core = 4
DG = HPC * HD  # 512 cols per head group
P = 128
SB = 512  # s-block (matmul free dim)
NSB = S // SB  # 4
NDT = D // P  # 16 contraction tiles of the model dim
NST = S // P  # 16 seq tiles
F32 = mybir.dt.float32
MMDT = mybir.dt.float16
MMNP = np.float16
SCALE = 1.0 / float(np.sqrt(HD))
EXP = mybir.ActivationFunctionType.Exp


def _bc_heads(ap2d, nh=HPC):
    """[rows, cols] AP -> [rows, nh, cols] with a 0-stride head dim."""
    return ap2d.unsqueeze(1).broadcast_to((ap2d.shape[0], nh, ap2d.shape[1]))


def _build_program(with_qkv_bias: bool):
    nc = bacc.Bacc("TRN2", target_bir_lowering=False, debug=False,
                   num_devices=NCORES)
    # all big inputs pre-tiled partition-major on the host so each DMA is
    # one contiguous run per partition (few descriptors, full spray)
    xH = nc.dram_tensor("xH", [P, NSB, NDT, SB], MMDT,
                        kind="ExternalInput").ap()
    wq = nc.dram_tensor("wq", [P, NDT, DG], MMDT, kind="ExternalInput").ap()
    wk = nc.dram_tensor("wk", [P, NDT, DG], MMDT, kind="ExternalInput").ap()
    wv = nc.dram_tensor("wv", [P, NDT, DG], MMDT, kind="ExternalInput").ap()
    wo = nc.dram_tensor("wo", [P, HPC, D], MMDT, kind="ExternalInput").ap()
    cosT = nc.dram_tensor("cosT", [P, S], MMDT, kind="ExternalInput").ap()
    sinST = nc.dram_tensor("sinST", [P, S], MMDT, kind="ExternalInput").ap()
    mask128 = nc.dram_tensor("mask128", [P, P], MMDT,
                             kind="ExternalInput").ap()
    onesd = nc.dram_tensor("ones", [P, P], MMDT, kind="ExternalInput").ap()
    if with_qkv_bias:
        bqr = nc.dram_tensor("bqrope", [P, HPC, S], F32,
                             kind="ExternalInput").ap()
        bkr = nc.dram_tensor("bkrope", [P, HPC, S], F32,
                             kind="ExternalInput").ap()
        bv128 = nc.dram_tensor("bv128", [P, DG], F32,
                               kind="ExternalInput").ap()
    out = nc.dram_tensor("out", [S, D], MMDT, kind="ExternalOutput").ap()

    with tile.TileContext(nc) as tc:
        with ExitStack() as top:
            # ---- persistent tiles ----
            qkT_pool = top.enter_context(tc.tile_pool(name="qkT", bufs=1))
            qT = qkT_pool.tile([P, HPC, S], MMDT, tag="qT")
            kT = qkT_pool.tile([P, HPC, S], MMDT, tag="kT")
            v_pool = top.enter_context(tc.tile_pool(name="vp", bufs=1))
            vN = v_pool.tile([P, NST, DG], MMDT, tag="vN")
            oT_pool = top.enter_context(tc.tile_pool(name="oTp", bufs=1))
            oT = oT_pool.tile([P, HPC, S], MMDT, tag="oT")
            wopool = top.enter_context(tc.tile_pool(name="wo", bufs=1))
            wo_t = wopool.tile([P, HPC, D], MMDT, tag="wo")
            cpool = top.enter_context(tc.tile_pool(name="cs", bufs=1))
            cos_t = cpool.tile([P, S], MMDT, tag="cos")
            sin_t = cpool.tile([P, S], MMDT, tag="sin")
            mpool = top.enter_context(tc.tile_pool(name="msk", bufs=1))
            mask_t = mpool.tile([P, P], MMDT, tag="mask")
            ones_t = mpool.tile([P, P], MMDT, tag="ones")

            # ---- phase A: q/k/v projections (sb3's v/k waves deferred
            # into attention as PE filler for its ACT-bound stretches) ----
            defer_vk = False
            a_sbuf = top if defer_vk else ExitStack()
            wpool = a_sbuf.enter_context(tc.tile_pool(name="wqkv", bufs=1))
            wq_t = wpool.tile([P, NDT, DG], MMDT, tag="wq")
            wk_t = wpool.tile([P, NDT, DG], MMDT, tag="wk")
            wv_t = wpool.tile([P, NDT, DG], MMDT, tag="wv")
            xpool = a_sbuf.enter_context(tc.tile_pool(name="xs", bufs=2))
            pcpool = a_sbuf.enter_context(tc.tile_pool(name="pc", bufs=2))
            tmppool = a_sbuf.enter_context(tc.tile_pool(name="rt", bufs=2))
            if with_qkv_bias:
                bpool = a_sbuf.enter_context(tc.tile_pool(name="bqk", bufs=1))
                bqr_t = bpool.tile([P, HPC, S], F32, tag="bqr")
                bkr_t = bpool.tile([P, HPC, S], F32, tag="bkr")
                bv_t = bpool.tile([P, DG], F32, tag="bv")

            # DMA launches in consumption order; the sync ring streams the
            # weights while the scalar ring streams x and the tables
            xs0 = xpool.tile([P, NDT, SB], MMDT, tag="xs", name="x_0")
            xs1 = xpool.tile([P, NDT, SB], MMDT, tag="xs", name="x_1")
            nc.sync.dma_start(wq_t[:, 0:4, :], wq[:, 0:4, :])
            nc.sync.dma_start(wq_t[:, 4:10, :], wq[:, 4:10, :])
            nc.sync.dma_start(wq_t[:, 10:16, :], wq[:, 10:16, :])
            nc.sync.dma_start(wk_t[:, 0:8, :], wk[:, 0:8, :])
            nc.sync.dma_start(wk_t[:, 8:16, :], wk[:, 8:16, :])
            nc.sync.dma_start(wv_t[:, 0:8, :], wv[:, 0:8, :])
            nc.sync.dma_start(wv_t[:, 8:16, :], wv[:, 8:16, :])
            nc.scalar.dma_start(xs0[:, 0:4, :], xH[:, 0, 0:4, :])
            nc.scalar.dma_start(xs0[:, 4:10, :], xH[:, 0, 4:10, :])
            nc.scalar.dma_start(xs0[:, 10:16, :], xH[:, 0, 10:16, :])
            nc.scalar.dma_start(cos_t[:], cosT[:])
            nc.scalar.dma_start(sin_t[:], sinST[:])
            nc.scalar.dma_start(mask_t[:], mask128[:])
            nc.scalar.dma_start(ones_t[:], onesd[:])
            nc.scalar.dma_start(xs1[:], xH[:, 1, :, :])
            nc.scalar.dma_start(wo_t[:], wo[:])
            if with_qkv_bias:
                nc.scalar.dma_start(bqr_t[:], bqr[:])
                nc.scalar.dma_start(bkr_t[:], bkr[:])
                nc.scalar.dma_start(bv_t[:], bv128[:])

            def rope(pc2d, dst_ap, ssl, bias_ap):
                """dst = pc*cos + rot(pc)*sinS via the pre-rotated sin table
                (both SB operands of each mul share a base partition)."""
                tmp = tmppool.tile([P, HPC, SB], MMDT, tag="rt")
                nc.vector.tensor_mul(tmp[0:64, 0, :], pc2d[64:128, :],
                                     sin_t[64:128, ssl])
                nc.vector.tensor_mul(tmp[64:128, 0, :], pc2d[0:64, :],
                                     sin_t[0:64, ssl])
                nc.vector.tensor_mul(dst_ap, pc2d[:], cos_t[:, ssl])
                nc.vector.tensor_add(dst_ap, dst_ap, tmp[:, 0, :])
                if bias_ap is not None:
                    nc.vector.tensor_add(dst_ap, dst_ap, bias_ap)

            with ExitStack() as actx:
                psA = actx.enter_context(
                    tc.tile_pool(name="psA", bufs=2, space="PSUM"))

                def qk_wave(sb, xs, wname, w_t, dst):
                    ssl = bass.ts(sb, SB)
                    ps = psA.tile([P, HPC, SB], F32, tag="psA",
                                  name=f"ps{wname}_{sb}")
                    for dt in range(NDT):
                        for h in range(HPC):
                            nc.tensor.matmul(
                                ps[:, h, :], w_t[:, dt, bass.ts(h, P)],
                                xs[:, dt, :],
                                start=(dt == 0), stop=(dt == NDT - 1))
                    pc = pcpool.tile([P, HPC, SB], MMDT, tag="pc")
                    nc.scalar.copy(pc[:], ps[:])
                    tmp = tmppool.tile([P, HPC, SB], MMDT, tag="rt")
                    nc.vector.tensor_mul(tmp[0:64, :, :], pc[64:128, :, :],
                                         _bc_heads(sin_t[64:128, ssl]))
                    nc.vector.tensor_mul(tmp[64:128, :, :], pc[0:64, :, :],
                                         _bc_heads(sin_t[0:64, ssl]))
                    dst_ap = dst[:, :, ssl]
                    nc.vector.tensor_mul(dst_ap, pc[:],
                                         _bc_heads(cos_t[:, ssl]))
                    nc.vector.tensor_add(dst_ap, dst_ap, tmp[:])
                    if with_qkv_bias:
                        bt = bqr_t if wname == "q" else bkr_t
                        nc.vector.tensor_add(dst_ap, dst_ap, bt[:, :, ssl])

                def v_wave(sb, xs):
                    psv = psA.tile([P, HPC, DG], F32, tag="psA",
                                   name=f"psv_{sb}")
                    for dt in range(NDT):
                        for j in range(4):
                            nc.tensor.matmul(
                                psv[:, j, :], xs[:, dt, bass.ts(j, P)],
                                wv_t[:, dt, :],
                                start=(dt == 0), stop=(dt == NDT - 1))
                    vdst = vN[:, bass.ts(sb, 4), :]
                    if sb == NSB - 1:
                        # last A-wave eviction on DVE so ACT is free for
                        # attention's first exps right at phase-A end
                        nc.vector.tensor_copy(vdst, psv[:])
                    else:
                        nc.scalar.copy(vdst, psv[:])
                    if with_qkv_bias:
                        nc.vector.tensor_add(
                            vdst, vdst,
                            bv_t[:].unsqueeze(1).broadcast_to((P, 4, DG)))

                n_full = NSB - 1 if defer_vk else NSB
                xs_tiles = {0: xs0, 1: xs1}
                for sb in range(NSB):
                    xs = xs_tiles.pop(sb)
                    xs_tiles[sb] = xs
                    if sb >= 1 and sb + 1 < NSB:
                        xn = xpool.tile([P, NDT, SB], MMDT, tag="xs",
                                        name=f"x_{sb + 1}")
                        nc.sync.dma_start(xn[:], xH[:, sb + 1, :, :])
                        xs_tiles[sb + 1] = xn
                    if sb < n_full:
                        qk_wave(sb, xs, "q", wq_t, qT)
                        qk_wave(sb, xs, "k", wk_t, kT)
                        v_wave(sb, xs)
                xs3 = xs_tiles[NSB - 1]

            if not defer_vk:
                # nothing in attention needs the A-phase SBUF pools
                a_sbuf.close()

            # ---- phase C+D: causal attention, with the deferred sb3 v/k
            # waves and the out-projection spliced in as PE filler ----
            with ExitStack() as cctx:
                etpool = cctx.enter_context(tc.tile_pool(name="et",
                                                          bufs=12))
                espool = cctx.enter_context(tc.tile_pool(name="es", bufs=8))
                rcpool = cctx.enter_context(tc.tile_pool(name="rc", bufs=4))
                obpool = cctx.enter_context(tc.tile_pool(name="ob", bufs=8))
                # psV first: it lands on the banks of A's final (v2) wave
                # whose gpsimd eviction clears shortly after A ends; psS
                # gets k2's banks which are free immediately
                psV = cctx.enter_context(
                    tc.tile_pool(name="psV", bufs=4, space="PSUM"))
                psS = cctx.enter_context(
                    tc.tile_pool(name="psS", bufs=2, space="PSUM"))

                pending = []
                evict_tog = [0]

                def v3_chunk(j):
                    def run():
                        psv = psV.tile([P, DG], F32, tag="psV",
                                       name=f"psv3_{j}")
                        for dt in range(NDT):
                            nc.tensor.matmul(
                                psv[:], xs3[:, dt, bass.ts(j, P)],
                                wv_t[:, dt, :],
                                start=(dt == 0), stop=(dt == NDT - 1))
                        nc.vector.tensor_copy(vN[:, 4 * (NSB - 1) + j, :],
                                              psv[:])
                    return run

                def qk3_chunk(h, w_t, dstT, nm):
                    def run():
                        ssl = bass.ts(NSB - 1, SB)
                        psk = psV.tile([P, SB], F32, tag="psV",
                                       name=f"ps{nm}3_{h}")
                        for dt in range(NDT):
                            nc.tensor.matmul(
                                psk[:], w_t[:, dt, bass.ts(h, P)],
                                xs3[:, dt, :],
                                start=(dt == 0), stop=(dt == NDT - 1))
                        pc = pcpool.tile([P, HPC, SB], MMDT, tag="pc")
                        nc.scalar.copy(pc[:, 0, :], psk[:])
                        rope(pc[:, 0, :], dstT[:, h, ssl], ssl, None)
                    return run

                if defer_vk:
                    for j in range(4):
                        pending.append(v3_chunk(j))
                        pending.append(qk3_chunk(j, wk_t, kT, "k"))
                    for h in range(HPC):
                        pending.append(qk3_chunk(h, wq_t, qT, "q"))

                def d_group(st, eb):
                    def run(act_evict=False):
                        pe = psV.tile([P, SB], F32, tag="psV",
                                      name=f"pe_{st}_{eb}")
                        for hh in range(HPC):
                            nc.tensor.matmul(
                                pe[:], oT[:, hh, bass.ts(st, P)],
                                wo_t[:, hh, bass.ts(eb, SB)],
                                start=(hh == 0), stop=(hh == HPC - 1))
                        # eviction on DVE: the ACT queue must stay clear
                        # for exps (attention's binding engine); the final
                        # drain alternates onto the then-idle ACT
                        ob = obpool.tile([P, SB], MMDT, tag="ob")
                        if act_evict:
                            nc.scalar.copy(ob[:], pe[:])
                        else:
                            nc.vector.tensor_copy(ob[:], pe[:])
                        nc.sync.dma_start(
                            out[bass.ts(st, P), bass.ts(eb, SB)], ob[:])
                    return run

                def emit_d(n, act_alt=False):
                    for i in range(min(n, len(pending))):
                        pending.pop(0)(act_alt and i % 2 == 1)

                for ib in (0, 1, 2, 3):
                    isl = bass.ts(ib, SB)
                    npair = 2 * (ib + 1)
                    for h in range(HPC):
                        po = psV.tile([P, SB], F32, tag="psV",
                                      name=f"po_{h}_{ib}")
                        pd = psV.tile([P, SB], F32, tag="psV",
                                      name=f"pd_{h}_{ib}")

                        def scores_exp(pt, h=h, ib=ib):
                            pss = psS.tile([P, 2, SB], F32, tag="psS",
                                           name=f"pss_{h}_{ib}_{pt}")
                            et = etpool.tile([P, 2, SB], MMDT, tag="et",
                                             name=f"et_{h}_{ib}_{pt}")
                            for t in range(2):
                                jt = 2 * pt + t
                                o = jt - 4 * ib
                                od = o * P if o > 0 else 0
                                nc.tensor.matmul(
                                    pss[:, t, od:],
                                    kT[:, h, bass.ts(jt, P)],
                                    qT[:, h, ib * SB + od:(ib + 1) * SB],
                                    start=True, stop=True)
                            if 2 * pt + 1 - 4 * ib < 0:
                                nc.scalar.activation(et[:], pss[:], EXP,
                                                     scale=SCALE)
                            else:
                                for t in range(2):
                                    od = (2 * pt + t - 4 * ib) * P
                                    nc.scalar.activation(et[:, t, od:],
                                                         pss[:, t, od:],
                                                         EXP, scale=SCALE)
                                    if od:
                                        nc.vector.memset(et[:, t, 0:od], 0.0)
                                    nc.vector.tensor_mul(
                                        et[:, t, od:od + P],
                                        et[:, t, od:od + P], mask_t[:])
                            es = espool.tile([P, SB], MMDT, tag="es")
                            nc.vector.tensor_add(es[:], et[:, 0, :],
                                                 et[:, 1, :])
                            return et, es

                        def consume(pt, et, es, prev_es, first, last,
                                    h=h, ib=ib):
                            for t in range(2):
                                jt = 2 * pt + t
                                o = jt - 4 * ib
                                od = o * P if o > 0 else 0
                                nc.tensor.matmul(
                                    po[:, od:], vN[:, jt, bass.ts(h, P)],
                                    et[:, t, od:],
                                    start=(first and t == 0),
                                    stop=(last and t == 1))
                            # one ones-matmul per TWO pairs: fold the earlier
                            # pair's sum into this one on DVE first
                            if pt % 2 == 1:
                                nc.vector.tensor_add(es[:], es[:], prev_es[:])
                                nc.tensor.matmul(pd[:], ones_t[:], es[:],
                                                 start=(pt == 1), stop=last)

                        data = {0: scores_exp(0)}
                        prev_es = None
                        for pt in range(npair):
                            if pt + 1 < npair:
                                data[pt + 1] = scores_exp(pt + 1)
                            et, es = data.pop(pt)
                            consume(pt, et, es, prev_es,
                                    pt == 0, pt == npair - 1)
                            prev_es = es

                        # pd is already partition-replicated by the
                        # ones-matmul, so the reciprocal runs on all 128
                        # partitions directly (no broadcast needed)
                        recs = rcpool.tile([P, SB], F32, tag="rc")
                        nc.vector.reciprocal_approx_fast(recs[:], pd[:])
                        nc.vector.tensor_mul(oT[:, h, isl], po[:], recs[:])
                        # only 2 groups here: psE then never reuses po/pd's
                        # PSUM bufs within the batch, so no WAR stall on the
                        # recip/norm/eviction DVE chain
                        emit_d(2)
                    # drain the leftovers PE-solid at the ib boundary
                    emit_d(len(pending))
                    for st in range(4 * ib, 4 * ib + 4):
                        for eb in range(NSB):
                            pending.append(d_group(st, eb))
                emit_d(len(pending), act_alt=True)

    nc.compile()
    return nc


def _rot_cols(w):
    """rotate_half applied to the last axis (head-dim columns) of w."""
    r = np.empty_like(w)
    r[..., : HD // 2] = -w[..., HD // 2:]
    r[..., HD // 2:] = w[..., : HD // 2]
    return r


def _host_inputs(x, cos, sin, qkv_w, qkv_b, with_qkv_bias):
    """Build the 8 per-core input maps."""
    # signed sin, transposed: sinS[d] = -sin[d] for d<64 else +sin[d]
    sinS = sin.copy()
    sinS[:, : HD // 2] *= -1.0
    cosT = np.ascontiguousarray(cos.T).astype(MMNP)
    # partition-rotated: sinROT[p] = sinS.T[(p + 64) % 128]
    sinST = np.ascontiguousarray(np.roll(sinS.T, 64, axis=0)).astype(MMNP)
    jj = np.arange(P)[:, None]
    ii = np.arange(P)[None, :]
    mask128 = (jj <= ii).astype(MMNP)
    ones = np.ones((P, P), dtype=MMNP)

    def tile_w(w2d):
        # [D, DG] -> [P, NDT, DG] with w[dt*P+p, n] at [p, dt, n]
        return np.ascontiguousarray(
            w2d.reshape(NDT, P, DG).transpose(1, 0, 2)).astype(MMNP)

    # x tiled: xH[p, sb, dt, s'] = x[sb*SB+s', dt*P+p]
    xHb = [np.ascontiguousarray(
        x[b].reshape(NSB, SB, NDT, P).transpose(3, 0, 2, 1)).astype(MMNP)
        for b in range(B)]
    qkv_w16 = qkv_w
    in_maps = []
    for c in range(NCORES):
        b, g = divmod(c, 4)
        cols = slice(g * DG, (g + 1) * DG)
        im = {
            "xH": xHb[b],
            "wq": tile_w(qkv_w16[:, cols]),
            "wk": tile_w(qkv_w16[:, D:][:, cols]),
            "wv": tile_w(qkv_w16[:, 2 * D:][:, cols]),
            "wo": None,  # filled by caller (needs out_w)
            "cosT": cosT,
            "sinST": sinST,
            "mask128": mask128,
            "ones": ones,
        }
        if with_qkv_bias:
            bq = qkv_b[cols]
            bk = qkv_b[D:][cols]
            bv = qkv_b[2 * D:][cols]
            # roped bias, transposed per head: [HD, HPC, S]
            def rope_bias(bvec):
                r = np.empty((P, HPC, S), dtype=np.float32)
                for h in range(HPC):
                    bh = bvec[h * HD:(h + 1) * HD]  # [HD]
                    rb = _rot_cols(bh[None, :])[0]
                    # b*cos + rot(b)*sin, as [HD, S]
                    r[:, h, :] = (bh[None, :] * cos + rb[None, :] * sin).T
                return r
            im["bqrope"] = rope_bias(bq)
            im["bkrope"] = rope_bias(bk)
            im["bv128"] = np.tile(bv[None, :], (P, 1)).astype(np.float32)
        in_maps.append(im)
    return in_maps


_CACHED = {}


def _get_program(with_qkv_bias):
    if with_qkv_bias not in _CACHED:
        _CACHED[with_qkv_bias] = _build_program(with_qkv_bias)
    return _CACHED[with_qkv_bias]


def run_on_cores(in_maps, profile_dir=None):
    """Execute the prebuilt program on 8 cores; optionally capture NTFF."""
    from concourse import bass2jax
    with_qkv_bias = "bqrope" in in_maps[0]
    nc = _get_program(with_qkv_bias)
    if profile_dir is not None:
        from trn_agent_boot.trn_boot import _ntff_profile_via_ctypes
        hook = _ntff_profile_via_ctypes("/opt/axon/libaxon_pjrt.so")
        with hook(profile_dir, [0]):
            results = bass2jax.run_bass_via_pjrt(nc, in_maps, n_cores=NCORES)
    else:
        results = bass2jax.run_bass_via_pjrt(nc, in_maps, n_cores=NCORES)
    return results


def kernel(x, cos, sin, qkv_w, qkv_b, out_w, out_b, _profile_dir=None):
    x = np.asarray(x, dtype=np.float32)
    cos = np.asarray(cos, dtype=np.float32)
    sin = np.asarray(sin, dtype=np.float32)
    qkv_w = np.asarray(qkv_w, dtype=np.float32)
    qkv_b = np.asarray(qkv_b, dtype=np.float32)
    out_w = np.asarray(out_w, dtype=np.float32)
    out_b = np.asarray(out_b, dtype=np.float32)

    with_qkv_bias = bool(np.any(qkv_b != 0))
    in_maps = _host_inputs(x, cos, sin, qkv_w, qkv_b, with_qkv_bias)
    for c in range(NCORES):
        g = c % 4
        # [DG, D] -> [P, HPC, D] with wo[h*P+p, d] at [p, h, d]
        in_maps[c]["wo"] = np.ascontiguousarray(
            out_w[g * DG:(g + 1) * DG, :].reshape(HPC, P, D)
            .transpose(1, 0, 2)).astype(MMNP)

    results = run_on_cores(in_maps, profile_dir=_profile_dir)

    final = np.zeros((B, S, D), dtype=np.float32)
    for c in range(NCORES):
        b = c // 4
        final[b] += results[c]["out"]
    final += out_b[None, None, :]
    return final
